# revision 61
# baseline (speedup 1.0000x reference)
"""Bass kernel for nn_Decoder (ragged tree-node decoder head), v2.

Per core (tokens = flattened (b,s,n), tokens-on-partitions layout):
  x   = G[feat_idx] + memrep[t]            (dma_gather + plain DMA + add)
  h1  = gelu(LN(x) @ W1' + cb1)
  h2  = gelu(LN(h1) @ W2' + cb2)
  p   = softmax(h2 @ W_out)

Key tricks vs v1 baseline:
  - G = gelu(emb @ W_feats + b_feats), W' = diag(ln_g) W, col-sums and
    cb = ln_b @ W' + b are all weight-only -> precomputed on host.
  - memory rows replicated per-token on host -> xm is a plain DMA, not a
    gpsimd gather (halves gpsimd load).
  - LN stats via one bn_stats/bn_aggr pass (no Square passes, no
    accumulator reads, no sqrt/gelu ACT-table thrash).
  - LN apply folded into the matmul: z = x@W' + [-mu; sd]@[wsum; cb]
    (rank-1 PSUM accumulate), gelu applied as ACT(z * rstd) with
    per-partition scale. Removes all big LN tensor_scalar ops.
Supergroup phasing keeps ACT table sets batched (sqrt / gelu / exp).
"""

import math
from contextlib import ExitStack

import numpy as np

import concourse.bass as bass
from concourse import bacc
import concourse.mybir as mybir
import concourse.tile as tile
from concourse.masks import make_identity

F32 = mybir.dt.float32
BF16 = mybir.dt.bfloat16
I16 = mybir.dt.int16
AF = mybir.ActivationFunctionType
ALU = mybir.AluOpType

D = 256
V = 64
N_NODES = 31
NKB = D // 128  # 2 contraction blocks


def build_nc(T, VE, SG, TILE=512):
    """T tokens on this core, VE embedding rows, SG tiles per supergroup,
    TILE tokens per tile (must be 4*128)."""
    NSUB = TILE // 128
    NT = T // TILE
    assert T % TILE == 0 and T % 16 == 0
    nc = bacc.Bacc()

    memrep_d = nc.dram_tensor("memrep", [T, D], BF16, kind="ExternalInput")
    idxg_d = nc.dram_tensor("idxg", [128, T // 128], mybir.dt.int32, kind="ExternalInput")
    g_d = nc.dram_tensor("g16", [VE, D], BF16, kind="ExternalInput")
    wp1_d = nc.dram_tensor("wp1", [D, D], BF16, kind="ExternalInput")
    wp2_d = nc.dram_tensor("wp2", [D, D], BF16, kind="ExternalInput")
    rkr1_d = nc.dram_tensor("rkr1", [2, D], BF16, kind="ExternalInput")
    rkr2_d = nc.dram_tensor("rkr2", [2, D], BF16, kind="ExternalInput")
    wout16_d = nc.dram_tensor("wout16", [D, V], BF16, kind="ExternalInput")
    out_d = nc.dram_tensor("out", [T, V], F32, kind="ExternalOutput")

    with tile.TileContext(nc) as tc, ExitStack() as ctx:
        singles = ctx.enter_context(tc.tile_pool(name="singles", bufs=1))
        bigs = ctx.enter_context(tc.tile_pool(name="bigs", bufs=1))
        xwork = ctx.enter_context(tc.tile_pool(name="xwork", bufs=3))
        gpool = ctx.enter_context(tc.tile_pool(name="gpool", bufs=8))
        tpsum = ctx.enter_context(tc.tile_pool(name="tpsum", bufs=2, space="PSUM"))
        zpsum = ctx.enter_context(tc.tile_pool(name="zpsum", bufs=4, space="PSUM"))
        hpsum = ctx.enter_context(tc.tile_pool(name="hpsum", bufs=1, space="PSUM"))
        spsum = ctx.enter_context(tc.tile_pool(name="spsum", bufs=1, space="PSUM"))

        # ---------------- constants / weights ----------------
        ident = singles.tile([128, 128], BF16)
        make_identity(nc, ident)
        eps_sb = singles.tile([128, 1], F32)
        nc.vector.memset(eps_sb, 1e-5)

        wp_sb = []
        rkr_sb = []
        for li, (wp_d, rk_d) in enumerate(((wp1_d, rkr1_d), (wp2_d, rkr2_d))):
            wp = singles.tile([128, NKB, D], BF16, tag=f"wp{li}")
            nc.sync.dma_start(out=wp, in_=wp_d[:, :].rearrange("(k p) e -> p k e", p=128))
            wp_sb.append(wp)
            rk = singles.tile([2, D], BF16, tag=f"rk{li}")
            nc.sync.dma_start(out=rk, in_=rk_d[:, :])
            rkr_sb.append(rk)
        wout_sb = singles.tile([128, NKB, V], BF16)
        nc.sync.dma_start(out=wout_sb, in_=wout16_d[:, :].rearrange("(k p) e -> p k e", p=128))

        idxg_sb = bigs.tile([128, T // 128], mybir.dt.int32)
        nc.sync.dma_start(out=idxg_sb, in_=idxg_d[:, :])

        # ---------------- big supergroup buffers ----------------
        xbuf = bigs.tile([128, SG, NSUB, D], BF16)
        hbuf = bigs.tile([128, SG, NSUB, D], BF16)
        logits = bigs.tile([128, SG, NSUB, V], F32)
        mv1 = bigs.tile([128, SG * NSUB, 2], F32)   # (mean, var) from bn_aggr
        mv2 = bigs.tile([128, SG * NSUB, 2], F32)
        st1 = bigs.tile([128, SG * NSUB, 2], BF16)  # (negmu, sd) for rank-1 rows
        st2 = bigs.tile([128, SG * NSUB, 2], BF16)
        sd32 = bigs.tile([128, SG * NSUB], F32)
        rstd1 = bigs.tile([128, SG * NSUB], F32)
        rstd2 = bigs.tile([128, SG * NSUB], F32)
        strow1 = bigs.tile([2, SG, NSUB, 128], BF16)  # transposed stat rows
        strow2 = bigs.tile([2, SG, NSUB, 128], BF16)
        # scalar-queue phase-ordering deps: supergroup k's gelus/sqrts wait on
        # these tiles, rewritten after supergroup k-1's exps -> no ACT-table
        # ping-pong between exp and gelu across supergroups.
        depz = bigs.tile([128, 1], F32)
        depone = bigs.tile([128, 1], F32)
        depacc = bigs.tile([128, 1], F32)
        deponeE = bigs.tile([128, 1], F32)
        depaccE = bigs.tile([128, 1], F32)
        nc.vector.memset(depz, 0.0)
        nc.vector.memset(depone, 1.0)

        # staggered supergroup sizes: small head groups so PE compute starts
        # while the bulk of the gathers stream in, large tail groups to keep
        # ACT-table switches rare
        ramp = [2, 4, 8, 16]
        tail = [6, 3]
        mid = NT - sum(ramp) - sum(tail)
        sg_sizes = list(ramp)
        while mid > 0:
            s = min(SG, mid)
            sg_sizes.append(s)
            mid -= s
        sg_sizes += tail
        assert sum(sg_sizes) == NT
        sg_bounds = []
        _t = 0
        for s in sg_sizes:
            sg_bounds.append(range(_t, _t + s))
            _t += s

        def tile_stats(src, src_ti, mv, ti):
            """bn_stats/bn_aggr: per-subtile (mean, var) of src tile."""
            bns = xwork.tile([128, NSUB, 6], F32, tag="bns")
            for j in range(NSUB):
                nc.vector.bn_stats(out=bns[:, j, :], in_=src[:, src_ti, j, :])
            for j in range(NSUB):
                nc.vector.bn_aggr(out=mv[:, ti * NSUB + j, :], in_=bns[:, j, :])

        def stats_finish(mv, st, rstd, strow, tiles, t0):
            """var -> sd, rstd; pack (negmu, sd) and transpose to rows."""
            sl = slice((tiles.start - t0) * NSUB, (tiles.stop - t0) * NSUB)
            nc.scalar.activation(
                out=sd32[:, sl], in_=mv[:, sl, 1], func=AF.Sqrt, bias=eps_sb,
                scale=depone,
            )
            nc.vector.reciprocal(out=rstd[:, sl], in_=sd32[:, sl])
            nc.vector.tensor_scalar_mul(out=st[:, sl, 0], in0=mv[:, sl, 0], scalar1=-1.0)
            nc.vector.tensor_copy(out=st[:, sl, 1], in_=sd32[:, sl])
            for t in range(tiles.start, tiles.stop):
                ti = t - t0
                for j in range(NSUB):
                    stps = spsum.tile([2, 128], BF16, tag="stps")
                    nc.tensor.transpose(stps, st[:, ti * NSUB + j, :], ident)
                    nc.vector.tensor_copy(out=strow[:, ti, j, :], in_=stps)

        def layer_tile(src, strow, rstd, ti, wp, rkr, dst, dst_ti, acc=None):
            """gelu((LN-folded src) @ W' + cb) -> dst."""
            xt = []
            for k in range(NKB):
                tps = tpsum.tile([128, TILE], BF16, tag="tps")
                for j in range(NSUB):
                    nc.tensor.transpose(
                        tps[:, j * 128 : (j + 1) * 128],
                        src[:, ti, j, k * 128 : (k + 1) * 128],
                        ident,
                    )
                xtk = xwork.tile([128, TILE], BF16, tag="xt")
                if k == 0:
                    nc.scalar.copy(out=xtk, in_=tps)
                else:
                    nc.vector.tensor_copy(xtk, tps)
                xt.append(xtk)
            for j in range(NSUB):
                jj = ti * NSUB + j
                z = zpsum.tile([128, D], F32, tag="z")
                for k in range(NKB):
                    nc.tensor.matmul(
                        z,
                        xt[k][:, j * 128 : (j + 1) * 128],
                        wp[:, k, :],
                        start=(k == 0),
                        stop=False,
                    )
                nc.tensor.matmul(
                    z,
                    strow[:, ti, j, :],
                    rkr,
                    start=False,
                    stop=True,
                )
                nc.scalar.activation(
                    out=dst[:, dst_ti, j, :],
                    in_=z,
                    func=AF.Gelu,
                    scale=rstd[:, jj : jj + 1],
                    bias=depz,
                    accum_out=(acc if j == NSUB - 1 else None),
                )

        for tiles in sg_bounds:
            t0 = tiles.start

            # -- phase A: gather + mem add + layer-1 stats (vector only) --
            for t in tiles:
                ti = t - t0
                xg = gpool.tile([128, NSUB, D], BF16, tag="xg")
                xm = gpool.tile([128, NSUB, D], BF16, tag="xm")
                for j in range(NSUB):
                    nc.gpsimd.indirect_dma_start(
                        out=xg[:, j, :],
                        out_offset=None,
                        in_=g_d[:, :],
                        in_offset=bass.IndirectOffsetOnAxis(
                            ap=idxg_sb[:, t * NSUB + j : t * NSUB + j + 1],
                            axis=0,
                        ),
                    )
                nc.sync.dma_start(
                    out=xm,
                    in_=memrep_d[t * TILE : (t + 1) * TILE, :].rearrange(
                        "(j p) d -> p j d", p=128
                    ),
                )
                nc.vector.tensor_tensor(
                    out=xbuf[:, ti], in0=xg, in1=xm, op=ALU.add
                )
                tile_stats(xbuf, ti, mv1, ti)

            # -- phase B: rstd1 + stat rows (sqrt table) --
            stats_finish(mv1, st1, rstd1, strow1, tiles, t0)

            # -- phase C: layer 1 (gelu table) + layer-2 stats --
            for t in tiles:
                ti = t - t0
                layer_tile(xbuf, strow1, rstd1, ti, wp_sb[0], rkr_sb[0], hbuf, ti)
                tile_stats(hbuf, ti, mv2, ti)

            # -- phase D: rstd2 (sqrt table) --
            stats_finish(mv2, st2, rstd2, strow2, tiles, t0)

            # -- phase E: layer 2 + head (gelu table) --
            for t in tiles:
                ti = t - t0
                h2 = xwork.tile([128, 1, NSUB, D], BF16, tag="h2")
                layer_tile(hbuf, strow2, rstd2, ti, wp_sb[1], rkr_sb[1], h2, 0,
                           acc=(depaccE if t == tiles.stop - 1 else None))
                h2t = []
                for k in range(NKB):
                    tps = tpsum.tile([128, TILE], BF16, tag="tps")
                    for j in range(NSUB):
                        nc.tensor.transpose(
                            tps[:, j * 128 : (j + 1) * 128],
                            h2[:, 0, j, k * 128 : (k + 1) * 128],
                            ident,
                        )
                    ht = xwork.tile([128, TILE], BF16, tag="xt")
                    nc.vector.tensor_copy(ht, tps)
                    h2t.append(ht)
                l_ps = hpsum.tile([128, NSUB, V], F32, tag="lps")
                for j in range(NSUB):
                    for k in range(NKB):
                        nc.tensor.matmul(
                            l_ps[:, j, :],
                            h2t[k][:, j * 128 : (j + 1) * 128],
                            wout_sb[:, k, :],
                            start=(k == 0),
                            stop=(k == NKB - 1),
                        )
                nc.vector.tensor_copy(logits[:, ti, :, :], l_ps)

            # order phase-F exps after the last phase-E gelu on the scalar queue
            nc.scalar.activation(
                out=deponeE, in_=depaccE, func=AF.Copy, scale=0.0, bias=1.0
            )

            # -- phase F: softmax + store (exp table) --
            for t in tiles:
                ti = t - t0
                et = xwork.tile([128, NSUB, V], F32, tag="et")
                nc.scalar.activation(
                    out=et, in_=logits[:, ti, :, :], func=AF.Exp,
                    scale=deponeE,
                    accum_out=(depacc if t == tiles.stop - 1 else None),
                )
                den = xwork.tile([128, NSUB], F32, tag="den")
                nc.vector.tensor_reduce(
                    out=den, in_=et, axis=mybir.AxisListType.X, op=ALU.add
                )
                rd = xwork.tile([128, NSUB], F32, tag="rd")
                nc.vector.reciprocal(out=rd, in_=den)
                for j in range(NSUB):
                    nc.vector.tensor_scalar_mul(
                        out=et[:, j, :], in0=et[:, j, :], scalar1=rd[:, j : j + 1]
                    )
                nc.sync.dma_start(
                    out=out_d[t * TILE : (t + 1) * TILE, :].rearrange(
                        "(j p) v -> p j v", p=128
                    ),
                    in_=et,
                )

            # rewrite the phase-ordering dep tiles after this supergroup's
            # exps so the next supergroup's gelus/sqrts queue after them
            nc.scalar.activation(out=depz, in_=depacc, func=AF.Copy, scale=0.0)
            nc.scalar.activation(
                out=depone, in_=depacc, func=AF.Copy, scale=0.0, bias=1.0
            )
    return nc


def wrap_idx(flat_idx):
    """dma_gather index layout: idx i -> (partition i%16, col i//16),
    replicated to all 8 q7 core groups."""
    base = np.asarray(flat_idx, dtype=np.int16).reshape(-1, 16).T  # [16, n/16]
    return np.tile(base, (8, 1)).copy()  # [128, n/16]


def _gelu(x):
    from scipy.special import erf

    return 0.5 * x * (1.0 + erf(x / np.sqrt(2.0)))


def host_prep(memory, feat_idx, emb, W_feats, b_feats, ln_g, ln_b, W1, b1, W2, b2,
              W_out, n_cores=8):
    """Per-core input maps. memory [BSall, D], feat_idx [BSall, N] flat (b,s).
    All weight-only terms precomputed here."""
    import ml_dtypes

    bs_all = memory.shape[0]
    n_nodes = feat_idx.shape[1]
    bs_c = bs_all // n_cores
    t = bs_c * n_nodes

    g = _gelu(emb.astype(np.float64) @ W_feats.astype(np.float64) + b_feats)
    wp1 = ln_g[:, None] * W1
    wp2 = ln_g[:, None] * W2
    rkr1 = np.stack([wp1.sum(0), ln_b @ wp1 + b1])
    rkr2 = np.stack([wp2.sum(0), ln_b @ wp2 + b2])
    shared = dict(
        g16=g.astype(ml_dtypes.bfloat16),
        wp1=wp1.astype(ml_dtypes.bfloat16),
        wp2=wp2.astype(ml_dtypes.bfloat16),
        rkr1=rkr1.astype(ml_dtypes.bfloat16),
        rkr2=rkr2.astype(ml_dtypes.bfloat16),
        wout16=W_out.astype(ml_dtypes.bfloat16),
    )
    in_maps = []
    for c in range(n_cores):
        mem_c = memory[c * bs_c : (c + 1) * bs_c].astype(ml_dtypes.bfloat16)
        memrep = np.repeat(mem_c, n_nodes, axis=0)  # [t, D]
        fi_c = feat_idx[c * bs_c : (c + 1) * bs_c].reshape(-1).astype(np.int32)
        # [128, NT*NSUB]: column (t*NSUB+j), partition p -> token t*512+j*128+p
        idxg = np.ascontiguousarray(
            fi_c.reshape(-1, 4, 128).transpose(2, 0, 1).reshape(128, -1)
        )
        in_maps.append(dict(shared, memrep=memrep, idxg=idxg))
    return in_maps


def run_full(inputs, trace=False):
    """inputs: dict from setup_inputs (full shapes). Returns (out, results_obj)."""
    from concourse.bass_utils import run_bass_kernel_spmd

    B_, S_, N_ = inputs["feat_idx"].shape
    D_ = inputs["memory"].shape[-1]
    n_cores = 8
    mem_flat = np.asarray(inputs["memory"], np.float32).reshape(B_ * S_, D_)
    fi_flat = np.asarray(inputs["feat_idx"]).reshape(B_ * S_, N_)
    in_maps = host_prep(
        mem_flat, fi_flat, np.asarray(inputs["emb"], np.float32),
        np.asarray(inputs["W_feats"], np.float32), np.asarray(inputs["b_feats"], np.float32),
        np.asarray(inputs["ln_g"], np.float32), np.asarray(inputs["ln_b"], np.float32),
        np.asarray(inputs["W1"], np.float32), np.asarray(inputs["b1"], np.float32),
        np.asarray(inputs["W2"], np.float32), np.asarray(inputs["b2"], np.float32),
        np.asarray(inputs["W_out"], np.float32), n_cores=n_cores,
    )
    bs_c = (B_ * S_) // n_cores
    t = bs_c * N_
    nc = build_nc(T=t, VE=inputs["emb"].shape[0], SG=21)
    nc.finalize()
    res = run_bass_kernel_spmd(nc, in_maps, list(range(n_cores)), trace=trace)
    out = np.concatenate([res.results[c]["out"] for c in range(n_cores)], axis=0)
    v = out.shape[-1]
    return out.reshape(B_, S_, N_, v), res


def kernel(**inputs):
    """Harness entry: full unsharded inputs -> full output [B,S,N,V] f32."""
    out, _ = run_full(inputs, trace=False)
    return out.astype(np.float32)
